# revision 22
# baseline (speedup 1.0000x reference)
"""Trainium2 Bass kernel for nn_CrossAttention (dense_transformer).  v3

Sharding: 8 cores = (batch b in 0..3) x (image half in 0..1).  Each core
computes its batch's half-image (64 rows + 1 halo row each side).  All
convs and the attention output are core-local; only the tiny per-head
Gram matrices and l2-norm square-sums are AllGather'd between the two
cores sharing a batch (replica groups [[0,1],[2,3],[4,5],[6,7]]).

v3 changes over v2 (from trace analysis):
  - DVE was the bottleneck (539us busy, mostly depthwise taps; the two
    odd-offset scalar_tensor_tensor taps ran with no DVE fast mode at
    2.4us each).  The odd taps move to the PE as extra diagonal-weight
    matmuls in the existing corner-tap psum chains (PE streams cols at
    1/cycle regardless of weights).
  - tap + conv PSUM switch to bf16 with 1024-col windows: halves the
    chain count and the psum-copy bytes.  Numerically equivalent to the
    old flow, which already accumulated taps in bf16 sbuf.
  - the ACT psum->acc copy is fused into the first DVE tap
    (scalar_tensor_tensor with in1 = bf16 psum), killing ~120us of ACT.
  - constant loads are batched into a few big DMAs.
  - attn@v psum is bf16 [96,1024] (one matmul per 8-row window).
"""
import numpy as np
import ml_dtypes

import concourse.bass as bass
import concourse.bacc as bacc
from concourse import mybir
from concourse.bass_utils import run_bass_kernel_spmd
from concourse.tile import TileContext

F32 = mybir.dt.float32
BF16 = mybir.dt.bfloat16
ALU = mybir.AluOpType
AF = mybir.ActivationFunctionType

C = 384          # channels
HEADS = 8
CP = C // HEADS  # 48
W = 128          # image width
CH = 16          # chunk center rows
CR = CH + 2      # conv rows per chunk
PADW = 132       # padded row stride: 2 pad, 128 img, 2 pad
PADN = CR * PADW             # padded buffer cols (2376)
OUT0, OUT1 = PADW, PADW + CH * PADW   # valid acc range [132, 2244)

# Feature flags (fallbacks for hw surprises)
# NOTE: TRN2 matmul psum must be fp32 (bf16 psum is TRN3-only).
PSUM_BF16_CONV = False
PSUM_BF16_DW = False
FUSE_INIT = True

# tap index = (dy+1)*3 + (dx+1); tap offset in pad layout = dy*PADW + dx
PE_TAPS = (0, 2, 6, 8, 3, 5)  # corners + (0,-1),(0,+1): diag matmuls
DVE_TS_TAPS = (1, 7)          # (dy=+-1, dx=0): tensor_scalar 4x + tt 2x
FUSED_TAP = 4                 # center tap: fused with psum init on DVE

CONV_WIN = 1024 if PSUM_BF16_CONV else 512
DW_WIN = 1024 if PSUM_BF16_DW else 512
CONV_PS_DT = BF16 if PSUM_BF16_CONV else F32
DW_PS_DT = BF16 if PSUM_BF16_DW else F32


def _tap_off(t):
    dy, dx = t // 3 - 1, t % 3 - 1
    return dy * PADW + dx


def _build(HALF):
    NCH = HALF // CH
    NSP = HALF * W
    NT = NSP // 128
    XSB = CR * W
    NNT = CH * W // 128

    nc = bacc.Bacc(num_devices=8)

    x_ext = nc.declare_dram_parameter("x", [C, HALF + 2, W], BF16, isOutput=False)
    y_ext = nc.declare_dram_parameter("y", [C, HALF + 2, W], BF16, isOutput=False)
    # packed 1x1 conv weights: [128, kt(3) * tensor(3) * 384]
    wpk_ext = nc.declare_dram_parameter("wpack", [128, 9 * C], BF16, isOutput=False)
    wpT_ext = nc.declare_dram_parameter("wpT", [4, 96, C], BF16, isOutput=False)
    # packed depthwise scalar weights: [128, tensor(3) * pt(4) * 9]
    dws_ext = nc.declare_dram_parameter("dwsc", [128, 3 * 4 * 9], F32, isOutput=False)
    # packed diagonal tap weights: [128, tensor(3) * tap(6) * pt(4) * 128]
    dg_ext = nc.declare_dram_parameter("dwdiag", [128, 3 * 6 * 4 * 128],
                                       BF16, isOutput=False)
    id_ext = nc.declare_dram_parameter("ident", [128, 128], BF16, isOutput=False)
    mask_ext = nc.declare_dram_parameter("blkmask", [96, 4 * 96], BF16,
                                         isOutput=False)
    tT_ext = nc.declare_dram_parameter("tempT", [1, C], F32, isOutput=False)
    out_ext = nc.declare_dram_parameter("out", [C, NSP], F32, isOutput=True)

    SLEN = 128 * 480
    CCN = SLEN + 2 * C
    cc_in = nc.dram_tensor("cc_in", [1, CCN], F32)
    cc_out = nc.dram_tensor("cc_out", [2, CCN], F32)
    rn_scr = nc.dram_tensor("rn_scr", [2, C], F32)

    with TileContext(nc) as tc:
        with tc.tile_pool(name="const", bufs=1) as cpool:
            wpk_sb = cpool.tile([128, 9 * C], BF16, tag="wpk", name="wpk")
            nc.scalar.dma_start(out=wpk_sb[:], in_=wpk_ext[:])
            # slice helpers: weight tile for (tensor t, kt)
            def wslice(t, kt):
                o = (kt * 3 + t) * C
                return wpk_sb[:, o:o + C]
            wp_sb = [cpool.tile([96, C], BF16, tag=f"wp{k}", name=f"wp{k}") for k in range(4)]
            for k in range(4):
                nc.scalar.dma_start(out=wp_sb[k][:], in_=wpT_ext[k])
            dws_sb = cpool.tile([128, 3 * 4 * 9], F32, tag="dwsc", name="dwsc")
            nc.scalar.dma_start(out=dws_sb[:], in_=dws_ext[:])
            def dwslice(t, pt):
                o = (t * 4 + pt) * 9
                return dws_sb[:, o:o + 9]
            dg_sb = cpool.tile([128, 3 * 6 * 4 * 128], BF16, tag="dg", name="dg")
            nc.scalar.dma_start(out=dg_sb[:], in_=dg_ext[:])
            def dgslice(t, i, pt, M):
                o = ((t * 6 + i) * 4 + pt) * 128
                return dg_sb[0:M, o:o + M]
            id_sb = cpool.tile([128, 128], BF16, tag="ident", name="ident")
            nc.scalar.dma_start(out=id_sb[:], in_=id_ext[:])
            mask_sb = cpool.tile([96, 4 * 96], BF16, tag="mask", name="mask")
            nc.scalar.dma_start(out=mask_sb[:], in_=mask_ext[:])
            tT_sb = cpool.tile([1, C], F32, tag="tempT", name="tempT")
            nc.scalar.dma_start(out=tT_sb[:], in_=tT_ext[:])

            nsq_q = cpool.tile([128, 3, NCH], F32, tag="nsqq", name="nsqq")
            nsq_k = cpool.tile([128, 3, NCH], F32, tag="nsqk", name="nsqk")
            sp_sb = cpool.tile([128, 480], F32, tag="spsb", name="spsb")

            # ------------- chunk worker: conv1x1 + depthwise -------------
            def conv_dw(pool, psp, src_sb, wt, dwt, M, pt, acc, acc_c=None,
                        pb=2):
                """One (chunk, out-ptile).  acc: [M, PADN] bf16 pad-layout
                output tile (pads end up holding junk)."""
                apad = pool.tile([M, PADN + 8], BF16, tag="apad", name="apad")
                ap3 = apad[:, 0:PADN].rearrange("p (r c) -> p r c", c=PADW)
                nc.vector.memset(ap3[:, :, 0:2], 0.0)
                nc.vector.memset(ap3[:, :, 130:132], 0.0)
                nc.vector.memset(apad[:, PADN:], 0.0)
                dw_sc = dwslice(dwt, pt)[0:M, :]
                # conv1x1: bf16 matmuls into bf16 psum, 1024-col windows
                for w0 in range(0, XSB, CONV_WIN):
                    w1 = min(XSB, w0 + CONV_WIN)
                    cps = psp.tile([M, CONV_WIN], CONV_PS_DT, tag="convps",
                                   name="convps")
                    for kt in range(3):
                        nc.tensor.matmul(
                            cps[:, 0:w1 - w0],
                            wslice(wt, kt)[:, M * pt:M * pt + M],
                            src_sb[:, kt, w0:w1],
                            start=(kt == 0), stop=(kt == 2),
                        )
                    r0, r1 = w0 // W, w1 // W
                    nc.any.tensor_copy(
                        ap3[:, r0:r1, 2:130],
                        cps[:, 0:w1 - w0].rearrange("p (r c) -> p r c", c=W),
                    )
                # PE taps into bf16 psum, windows <= 1024; the psum partial
                # is folded into acc by the first (fused) DVE tap.
                # Windows start at OUT0+2 so corner taps never index < 0
                # (acc pad columns hold junk by design).
                o0 = OUT0 + 2
                while o0 < OUT1:
                    wlen = min(DW_WIN, OUT1 - o0)
                    dps = psp.tile([M, DW_WIN], DW_PS_DT, tag="dwps", name="dwps",
                                   bufs=pb)
                    for i, t in enumerate(PE_TAPS):
                        nc.tensor.matmul(
                            dps[:, 0:wlen],
                            dgslice(dwt, i, pt, M),
                            apad[:, o0 + _tap_off(t):o0 + _tap_off(t) + wlen],
                            start=(i == 0), stop=(i == len(PE_TAPS) - 1),
                        )
                    if FUSE_INIT:
                        nc.vector.scalar_tensor_tensor(
                            out=acc[:, o0:o0 + wlen],
                            in0=apad[:, o0 + _tap_off(FUSED_TAP):
                                     o0 + _tap_off(FUSED_TAP) + wlen],
                            scalar=dw_sc[:, FUSED_TAP:FUSED_TAP + 1],
                            in1=dps[:, 0:wlen],
                            op0=ALU.mult, op1=ALU.add,
                        )
                    else:
                        nc.scalar.copy(acc[:, o0:o0 + wlen], dps[:, 0:wlen])
                    o0 += wlen
                if not FUSE_INIT:
                    nc.vector.scalar_tensor_tensor(
                        out=acc[:, OUT0:OUT1],
                        in0=apad[:, OUT0 + _tap_off(FUSED_TAP):
                                 OUT1 + _tap_off(FUSED_TAP)],
                        scalar=dw_sc[:, FUSED_TAP:FUSED_TAP + 1],
                        in1=acc[:, OUT0:OUT1],
                        op0=ALU.mult, op1=ALU.add,
                    )
                for ti, t in enumerate(DVE_TS_TAPS):
                    tmp = pool.tile([M, OUT1 - OUT0], BF16, tag="dwtmp", name="dwtmp")
                    nc.vector.tensor_scalar(
                        tmp[:], apad[:, OUT0 + _tap_off(t):OUT1 + _tap_off(t)],
                        dw_sc[:, t:t + 1], None, ALU.mult)
                    if acc_c is not None and ti == len(DVE_TS_TAPS) - 1:
                        # final tap writes the compact [M, CH*W] layout
                        # (same DVE cost; gives the xbar transpose a
                        # contiguous source)
                        nc.vector.tensor_tensor(
                            out=acc_c[:].rearrange("p (r c) -> p r c", c=W),
                            in0=tmp[:].rearrange(
                                "p (r c) -> p r c", c=PADW)[:, :, 2:130],
                            in1=acc[:, OUT0:OUT1].rearrange(
                                "p (r c) -> p r c", c=PADW)[:, :, 2:130],
                            op=ALU.add)
                    else:
                        nc.vector.tensor_tensor(
                            out=acc[:, OUT0:OUT1], in0=tmp[:],
                            in1=acc[:, OUT0:OUT1], op=ALU.add)
                return apad

            def load_chunk(pool, ext, ci, sbufs=2):
                t_ = pool.tile([128, 3, XSB], BF16, tag="src", name="src",
                               bufs=sbufs)
                for kt_ in range(3):
                    nc.sync.dma_start(
                        out=t_[:, kt_, :],
                        in_=ext[128 * kt_:128 * kt_ + 128,
                                CH * ci:CH * ci + CR, :],
                    )
                return t_

            # ======================= subpasses k, q ======================
            # Gram is computed in 128-row channel blocks; block pt needs
            # k-columns in window [KBASE[pt], KBASE[pt]+KWLEN[pt]) only
            # (the 48-wide head diagonal).  The 96-group softmax layout is
            # restored during the collective gather via piecewise APs.
            KBASE = (0, 96, 240)
            KWLEN = (144, 192, 144)
            with tc.tile_pool(name="ktpool", bufs=1) as ktp:
                kTp = [ktp.tile([128, NT, 128], BF16, tag=f"kTp{p}",
                                name=f"kTp{p}") for p in range(3)]

                with tc.tile_pool(name="sp1", bufs=2) as pool, \
                     tc.tile_pool(name="ps1", bufs=2, space="PSUM") as psp, \
                     tc.tile_pool(name="gramp", bufs=1, space="PSUM") as gramp:

                    s_ps = [gramp.tile([128, KWLEN[p]], F32, tag=f"sps{p}",
                                       name=f"sps{p}") for p in range(3)]
                    # (dst-pt, dst-col, lhsT-pt, k-pt, k-col0, k-col1)
                    GRAM_MM = (
                        (0, 0, 0, 0, 0, 128), (0, 128, 0, 1, 0, 16),
                        (1, 0, 1, 0, 96, 128), (1, 32, 1, 1, 0, 128),
                        (1, 160, 1, 2, 0, 32),
                        (2, 0, 2, 1, 112, 128), (2, 16, 2, 2, 0, 128),
                    )

                    def do_tensor(src, wt, nsq, ci, tr_dst):
                        for pt in range(3):
                            acc = pool.tile([128, PADN], BF16, tag="dwacc", name="dwacc")
                            acc_c = pool.tile([128, CH * W], BF16, tag="accc",
                                              name="accc")
                            apad = conv_dw(pool, psp, src, wt, wt,
                                           128, pt, acc[:], acc_c, pb=3)
                            nc.scalar.activation(
                                apad[:, 0:CH * W], acc_c[:], AF.Square,
                                accum_out=nsq[:, pt, ci:ci + 1],
                            )
                            nc.sync.dma_start_transpose(
                                out=tr_dst(pt), in_=acc_c[:])

                    for ci in range(NCH):
                        ysrc = load_chunk(pool, y_ext, ci, sbufs=3)
                        do_tensor(ysrc, 1, nsq_k, ci,
                                  lambda pt, _ci=ci: kTp[pt][
                                      :, NNT * _ci:NNT * (_ci + 1), :])

                    for ci in range(NCH):
                        xsrc = load_chunk(pool, x_ext, ci, sbufs=3)
                        qTp = [pool.tile([128, NNT, 128], BF16, bufs=2,
                                         tag=f"qTp{p}", name=f"qTp{p}")
                               for p in range(3)]
                        do_tensor(xsrc, 0, nsq_q, ci,
                                  lambda pt, _q=qTp: _q[pt][:])
                        for j in range(NNT):
                            gnt = NNT * ci + j
                            first = (ci == 0 and j == 0)
                            last = (ci == NCH - 1 and j == NNT - 1)
                            for (dp, dc, lp, kp, k0, k1) in GRAM_MM:
                                nc.tensor.matmul(
                                    s_ps[dp][:, dc:dc + k1 - k0],
                                    qTp[lp][:, j, :],
                                    kTp[kp][:, gnt, k0:k1],
                                    start=first, stop=last,
                                    skip_group_check=True,
                                )

                    for p in range(3):
                        o = sum(KWLEN[:p])
                        nc.scalar.copy(sp_sb[:, o:o + KWLEN[p]], s_ps[p][:])

            # ========== collective + softmax, then v-subpass with ==========
            # ========== per-chunk attn@v + proj interleaved        ==========
            nsqr_q = cpool.tile([128, 3], F32, tag="nsqrq", name="nsqrq")
            nsqr_k = cpool.tile([128, 3], F32, tag="nsqrk", name="nsqrk")
            nc.vector.tensor_reduce(out=nsqr_q[:], in_=nsq_q[:],
                                    axis=mybir.AxisListType.X, op=ALU.add)
            nc.vector.tensor_reduce(out=nsqr_k[:], in_=nsq_k[:],
                                    axis=mybir.AxisListType.X, op=ALU.add)

            ccs = [cpool.tile([96, 4 * 96], F32, tag=f"ccs{r}", name=f"ccs{r}") for r in range(2)]
            ccnq = [cpool.tile([128, 3], F32, tag=f"ccnq{r}", name=f"ccnq{r}") for r in range(2)]
            ccnk = [cpool.tile([128, 3], F32, tag=f"ccnk{r}", name=f"ccnk{r}") for r in range(2)]

            with tc.tile_pool(name="vstore", bufs=1) as vstp:
                vst = [vstp.tile([96, NCH, PADN], BF16, tag=f"vst{p}", name=f"vst{p}")
                       for p in range(4)]

                with tc.tile_pool(name="sp2", bufs=2) as pool, \
                     tc.tile_pool(name="ps2", bufs=2, space="PSUM") as psp, \
                     tc.tile_pool(name="smp", bufs=1) as smp, \
                     tc.tile_pool(name="p2", bufs=3) as p2:

                    # ---- v-subpass convs (issued before the collective
                    # so they fill the PE/DVE while gpsimd runs it) ----
                    for ci in range(NCH):
                        ysrc = load_chunk(pool, y_ext, ci)
                        for pt in range(4):
                            conv_dw(pool, psp, ysrc, 2, 2,
                                    96, pt, vst[pt][:, ci, :])

                    # ---- collective (gpsimd-only critical section) ----
                    # return gathers remap the [128-block x k-window] gram
                    # into the 96-group block-diagonal layout: piece list of
                    # (dst row0, row1, group g, src flat base, src rowlen).
                    # Entries outside a block's k-window read adjacent junk,
                    # which the blkmask zeroes later.
                    # staged layout: pos = p*480 + secoff[pt] + c' with
                    # secoff = (0, 144, 336); all pieces stride 480.
                    REMAP = (
                        (0, 96, 0, 0, 480),
                        (0, 32, 1, 96 * 480 + 96, 480),
                        (32, 96, 1, 144, 480),
                        (0, 64, 2, 64 * 480 + 240, 480),
                        (64, 96, 2, 288, 480),
                        (0, 96, 3, 32 * 480 + 384, 480),
                    )
                    with tc.tile_critical():
                        ccsem = nc.alloc_semaphore("ccsem")
                        sv = 0
                        nc.gpsimd.dma_start(
                            out=cc_in[0, 0:SLEN].rearrange("(p n) -> p n", p=128),
                            in_=sp_sb[:]).then_inc(ccsem, 16)
                        sv += 16
                        nc.gpsimd.dma_start(
                            out=cc_in[0, SLEN:SLEN + C].rearrange(
                                "(n p) -> p n", p=128),
                            in_=nsqr_q[:]).then_inc(ccsem, 16)
                        sv += 16
                        nc.gpsimd.dma_start(
                            out=cc_in[0, SLEN + C:].rearrange(
                                "(n p) -> p n", p=128),
                            in_=nsqr_k[:]).then_inc(ccsem, 16)
                        sv += 16
                        nc.gpsimd.wait_ge(ccsem, sv)
                        nc.gpsimd.collective_compute(
                            "AllGather", ALU.bypass,
                            replica_groups=[[0, 1], [2, 3], [4, 5], [6, 7]],
                            ins=[cc_in[:].opt()],
                            outs=[cc_out[:].opt()],
                        ).then_inc(ccsem, 1)
                        sv += 1
                        nc.gpsimd.wait_ge(ccsem, sv)
                        for r in range(2):
                            for (r0, r1, g, base, rl) in REMAP:
                                nc.gpsimd.dma_start(
                                    out=ccs[r][r0:r1, 96 * g:96 * g + 96],
                                    in_=cc_out[
                                        r, base:base + (r1 - r0) * rl
                                    ].rearrange("(a b) -> a b", b=rl)[:, 0:96],
                                ).then_inc(ccsem, 16)
                                sv += 16
                            nc.gpsimd.dma_start(
                                out=ccnq[r][:],
                                in_=cc_out[r, SLEN:SLEN + C].rearrange(
                                    "(n p) -> p n", p=128)).then_inc(ccsem, 16)
                            sv += 16
                            nc.gpsimd.dma_start(
                                out=ccnk[r][:],
                                in_=cc_out[r, SLEN + C:].rearrange(
                                    "(n p) -> p n", p=128)).then_inc(ccsem, 16)
                            sv += 16
                        nc.gpsimd.wait_ge(ccsem, sv)

                    # -------------------- softmax --------------------
                    s_full = smp.tile([96, 4, 96], F32, tag="sfull", name="sfull")
                    nc.vector.tensor_tensor(
                        out=s_full[:],
                        in0=ccs[0][:].rearrange("p (g n) -> p g n", n=96),
                        in1=ccs[1][:].rearrange("p (g n) -> p g n", n=96),
                        op=ALU.add)
                    rnq = smp.tile([128, 3], F32, tag="rnq", name="rnq")
                    rnk = smp.tile([128, 3], F32, tag="rnk", name="rnk")
                    nc.vector.tensor_tensor(out=rnq[:], in0=ccnq[0][:],
                                            in1=ccnq[1][:], op=ALU.add)
                    nc.vector.tensor_tensor(out=rnk[:], in0=ccnk[0][:],
                                            in1=ccnk[1][:], op=ALU.add)
                    nc.scalar.activation(rnq[:], rnq[:], AF.Sqrt)
                    nc.scalar.activation(rnk[:], rnk[:], AF.Sqrt)
                    nc.vector.tensor_scalar_max(rnq[:], rnq[:], 1e-12)
                    nc.vector.tensor_scalar_max(rnk[:], rnk[:], 1e-12)
                    nc.vector.reciprocal(rnq[:], rnq[:])
                    nc.vector.reciprocal(rnk[:], rnk[:])

                    rnqT = smp.tile([1, C], F32, tag="rnqT", name="rnqT")
                    rnkT = smp.tile([1, C], F32, tag="rnkT", name="rnkT")
                    with tc.tile_critical():
                        rsem = nc.alloc_semaphore("rsem")
                        nc.gpsimd.dma_start(
                            out=rn_scr[0, :].rearrange("(n p) -> p n", p=128),
                            in_=rnq[:]).then_inc(rsem, 16)
                        nc.gpsimd.dma_start(
                            out=rn_scr[1, :].rearrange("(n p) -> p n", p=128),
                            in_=rnk[:]).then_inc(rsem, 16)
                        nc.gpsimd.wait_ge(rsem, 32)
                        nc.gpsimd.dma_start(
                            out=rnqT[:], in_=rn_scr[0:1, :]).then_inc(rsem, 16)
                        nc.gpsimd.dma_start(
                            out=rnkT[:], in_=rn_scr[1:2, :]).then_inc(rsem, 16)
                        nc.gpsimd.wait_ge(rsem, 64)
                    nc.vector.tensor_tensor(out=rnqT[:], in0=rnqT[:],
                                            in1=tT_sb[:], op=ALU.mult)

                    outer_ps = psp.tile([96, 4, 96], F32, tag="outerps",
                                        name="outerps", bufs=1)
                    for p in range(4):
                        nc.tensor.matmul(
                            outer_ps[:, p, :],
                            rnqT[0:1, 96 * p:96 * p + 96],
                            rnkT[0:1, 96 * p:96 * p + 96],
                            start=True, stop=True,
                        )
                    logits = smp.tile([96, 4, 96], F32, tag="logits", name="logits")
                    nc.vector.tensor_tensor(out=logits[:], in0=s_full[:],
                                            in1=outer_ps[:], op=ALU.mult)
                    expv = smp.tile([96, 4 * 96], F32, tag="expv", name="expv")
                    nc.scalar.activation(
                        expv[:], logits[:].rearrange("p g n -> p (g n)"),
                        AF.Exp)
                    expm = smp.tile([96, 4, 96], F32, tag="expm", name="expm")
                    nc.vector.tensor_tensor(
                        out=expm[:],
                        in0=expv[:].rearrange("p (g n) -> p g n", n=96),
                        in1=mask_sb[:].rearrange("p (g n) -> p g n", n=96),
                        op=ALU.mult)
                    rs = smp.tile([96, 4], F32, tag="rs", name="rs")
                    nc.vector.tensor_reduce(out=rs[:], in_=expm[:],
                                            axis=mybir.AxisListType.X,
                                            op=ALU.add)
                    nc.vector.reciprocal(rs[:], rs[:])
                    attn = smp.tile([96, 4, 96], BF16, tag="attn", name="attn")
                    for p in range(4):
                        nc.vector.tensor_scalar(
                            attn[:, p, :], expm[:, p, :], rs[:, p:p + 1],
                            None, ALU.mult)
                    aT = smp.tile([96, 4, 96], BF16, tag="aT", name="aT")
                    for p in range(4):
                        atps = psp.tile([96, 96], BF16, tag="atps",
                                        name="atps", bufs=1)
                        nc.tensor.transpose(atps[:], attn[:, p, :],
                                            id_sb[0:96, 0:96])
                        nc.any.tensor_copy(aT[:, p, :], atps[:])

                    # ---- attn@v + proj, per chunk (scheduler overlaps
                    # with the v tail via vst chunk deps) ----
                    for ci in range(NCH):
                        for w2 in range(CH // 8):      # 8-row output windows
                            r0 = 1 + 8 * w2
                            ao = p2.tile([96, 4, 1024], BF16, tag="ao", name="ao")
                            for p in range(4):
                                for hf in range(2):
                                    avps = psp.tile([96, 512], F32,
                                                    tag="p2ps", name="avps")
                                    nc.tensor.matmul(
                                        avps[:].rearrange("p (r c) -> p r c", c=W),
                                        aT[:, p, :],
                                        vst[p][:, ci, :].rearrange(
                                            "p (r c) -> p r c", c=PADW)[
                                            :, r0 + 4 * hf:r0 + 4 * hf + 4, 2:130],
                                        start=True, stop=True)
                                    nc.any.tensor_copy(
                                        ao[:, p, 512 * hf:512 * hf + 512], avps[:])
                            for half in range(2):
                                hw0 = 512 * half
                                for o in range(3):
                                    pps = psp.tile([128, 512], F32,
                                                   tag="p2ps", name="pps")
                                    for kp in range(4):
                                        nc.tensor.matmul(
                                            pps[:],
                                            wp_sb[kp][:, 128 * o:128 * o + 128],
                                            ao[:, kp, hw0:hw0 + 512],
                                            start=(kp == 0), stop=(kp == 3))
                                    osb = p2.tile([128, 512], F32, tag="osb", name="osb")
                                    nc.any.tensor_copy(osb[:], pps[:])
                                    w2g = ci * (CH // 8) + w2
                                    nc.sync.dma_start(
                                        out=out_ext[
                                            128 * o:128 * o + 128,
                                            1024 * w2g + hw0:1024 * w2g + hw0 + 512],
                                        in_=osb[:])
    return nc


_BUILD_CACHE = {}


def _get_program(HALF):
    if HALF not in _BUILD_CACHE:
        nc = _build(HALF)
        if not nc.is_finalized():
            nc.finalize()
        _BUILD_CACHE[HALF] = nc
    return _BUILD_CACHE[HALF]


def kernel(x, y, Wq, Wkv, Wdw, Wproj, temperature):
    B, C_, H, W_ = x.shape
    assert C_ == C and W_ == W
    HALF = H // 2
    nc = _get_program(HALF)

    f32 = np.float32
    bf16 = ml_dtypes.bfloat16
    x = np.asarray(x, f32)
    y = np.asarray(y, f32)
    Wq = np.asarray(Wq, f32)
    Wkv = np.asarray(Wkv, f32)
    Wdw = np.asarray(Wdw, f32)
    Wproj = np.asarray(Wproj, f32)
    temperature = np.asarray(temperature, f32)

    wqT = np.ascontiguousarray(Wq.T)
    wkT = np.ascontiguousarray(Wkv[:C].T)
    wvT = np.ascontiguousarray(Wkv[C:].T)
    # packed conv weights [128, kt*tensor*384]: block (kt, t) holds
    # W_t^T[128*kt + p, :]
    wpack = np.zeros((128, 9 * C), f32)
    for kt in range(3):
        for t, wT in enumerate((wqT, wkT, wvT)):
            wpack[:, (kt * 3 + t) * C:(kt * 3 + t + 1) * C] = \
                wT[128 * kt:128 * kt + 128, :]
    wpack = wpack.astype(bf16)
    wpT = np.ascontiguousarray(Wproj.T.reshape(4, 96, C)).astype(bf16)
    dwq = np.ascontiguousarray(Wdw[0:C, 0].reshape(C, 9))
    dwk = np.ascontiguousarray(Wdw[C:2 * C, 0].reshape(C, 9))
    dwv = np.ascontiguousarray(Wdw[2 * C:, 0].reshape(C, 9))

    # packed per-channel tap scalars [128, t*4*9]
    dwsc = np.zeros((128, 3 * 4 * 9), f32)
    for t, dw in enumerate((dwq, dwk, dwv)):
        psz = 128 if t < 2 else 96
        npt = 3 if t < 2 else 4
        for pt in range(npt):
            dwsc[0:psz, (t * 4 + pt) * 9:(t * 4 + pt) * 9 + 9] = \
                dw[psz * pt:psz * pt + psz, :]

    # packed diagonal tap weights [128, t*6*4*128]
    npe = len(PE_TAPS)
    dgpack = np.zeros((128, 3 * 6 * 4 * 128), f32)
    for t, dw in enumerate((dwq, dwk, dwv)):
        psz = 128 if t < 2 else 96
        npt = 3 if t < 2 else 4
        for i, tap in enumerate(PE_TAPS):
            for pt in range(npt):
                o = ((t * 6 + i) * 4 + pt) * 128
                dgpack[np.arange(psz), o + np.arange(psz)] = \
                    dw[psz * pt:psz * pt + psz, tap]
    dgpack = dgpack.astype(bf16)

    ident = np.eye(128, dtype=bf16)
    blk = np.zeros((96, 4 * 96), f32)
    for p in range(4):
        blk[0:48, 96 * p:96 * p + 48] = 1.0
        blk[48:96, 96 * p + 48:96 * p + 96] = 1.0
    blkmask = blk.astype(bf16)
    tempT = np.repeat(temperature.reshape(HEADS), CP).reshape(1, C).astype(f32)

    in_maps = []
    for c in range(8):
        b, half = c // 2, c % 2
        r0 = half * HALF

        def shard(t):
            s = np.zeros((C, HALF + 2, W_), f32)
            s[:, 1:HALF + 1] = t[b, :, r0:r0 + HALF]
            if r0 > 0:
                s[:, 0] = t[b, :, r0 - 1]
            if r0 + HALF < H:
                s[:, HALF + 1] = t[b, :, r0 + HALF]
            return s.astype(bf16)

        in_maps.append({
            "x": shard(x), "y": shard(y),
            "wpack": wpack, "wpT": wpT, "dwsc": dwsc,
            "dwdiag": dgpack, "ident": ident, "blkmask": blkmask,
            "tempT": tempT,
        })

    import os
    trace = bool(os.environ.get("KBENCH_TRACE"))
    kw = {}
    if trace:
        kw = dict(trace=True)
    res = run_bass_kernel_spmd(nc, in_maps, list(range(8)), **kw)
    kernel._last_result = res

    out = np.zeros((B, C, H, W_), f32)
    for c in range(8):
        b, half = c // 2, c % 2
        out[b, :, half * HALF:(half + 1) * HALF] = \
            np.asarray(res.results[c]["out"], f32).reshape(C, HALF, W_)
    return out
